# revision 4
# baseline (speedup 1.0000x reference)
"""Point-Transformer block as a Bass/Tile kernel for 8 Trainium2 NeuronCores.

V2 strategy (see kernel.py docstring for the baseline algebra, all of which
is kept: x1/x2 folding into gate logits, ptf = v (x) ptsn collapse, deferred
b3 through the softmax).

New in V2:
  * bf16 everywhere off the accumulators: feats, xn, gates, y, all weights.
    Halves the feats DMA and enables cheap gate transport.
  * Compact gate exp: softmax-numerator exps run on [16*K, 256] compact
    layouts (free-size 256) instead of the 128-row broadcast [128, K*256]
    (free-size 2560) -- Act engine time for exp drops ~5x.
  * Gate broadcast for nD of the 10 k-slices moves to the DMA engines:
    compact e rows (gh*nD+k) -> DRAM -> replicate x8 (stride-0 DMA) ->
    affine re-read as ws [128, nD, 256] under the channel permutation
    m = gh*8 + cc (gh = (c%8)*2+h, cc = c//8).  The remaining nW = 10-nD
    k's keep the baseline wkat path (PE logit broadcast + Act exp).
  * ptf gated term folded post-gate:  sum_k ws_k*(v (x) ptsn_k)
      = v (x) sum_k e_k*ptsn_k, computed as eP = e .* ptsn_rep (vector op on
    compact rows) followed by two v-weighted selection matmuls accumulating
    straight into the num PSUM bank.  Kills the per-k vk matmuls.
  * k-reduction of y = ws .* x3 runs on the PE as 10 identity-matmul PSUM
    accumulations with strided rhs (plus the two vsel matmuls above).
  * b3 == 0 (asserted): relu commutes with the positive softmax scale, so
    normalization happens after the relu: o1p = relu(num) * rsb.
  * Gate pipeline for tile i+1 is emitted during tile i so the DRAM
    round-trip of the gate broadcast is hidden behind a full tile of work.
"""

import numpy as np
import ml_dtypes

B, N, K = 8, 8192, 10
CH = 64
SP = 8
TN = 512
TN2 = TN // 2
NT = N // TN

ND = 6            # k's transported via DMA broadcast (even, 2..6)
NW = K - ND       # k's via wkat (PE broadcast + Act exp)

_CACHE = {}

BF16 = ml_dtypes.bfloat16


def _gh(c, h):
    return (c % 8) * 2 + h


def _pi(c, h):
    return _gh(c, h) * 8 + c // 8


def _build_bass():
    import concourse.bacc as bacc
    import concourse.tile as tile
    from concourse import mybir

    f32 = mybir.dt.float32
    f32r = mybir.dt.float32r
    bf16 = mybir.dt.bfloat16
    AF = mybir.ActivationFunctionType
    OP = mybir.AluOpType

    nc = bacc.Bacc("TRN2", target_bir_lowering=False)

    def mm(out, lhsT, rhs, **kw):
        nc.tensor.matmul(out, lhsT, rhs, **kw)

    # ---------------- DRAM I/O ----------------
    feats_d = nc.dram_tensor("feats", [128, N * K // 2], bf16,
                             kind="ExternalInput")
    # pt1 rows: ptD (gh*ND+k); pt2: ptW (gh*NW+j); pt20: baseline (j + 10h)
    pt1_d = nc.dram_tensor("pt1", [NT, 16 * ND, TN2], bf16,
                           kind="ExternalInput")
    pt2_d = nc.dram_tensor("pt2", [NT, 16 * NW, TN2], bf16,
                           kind="ExternalInput")
    pt20_d = nc.dram_tensor("pt20", [NT, 20, TN2], bf16,
                            kind="ExternalInput")
    out_d = nc.dram_tensor("out", [CH, N], f32, kind="ExternalOutput")
    eflat_d = nc.dram_tensor("eflat", [NT, 16 * ND, TN2], bf16,
                             kind="Internal")
    e8x_d = nc.dram_tensor("e8x", [NT, 16, 8, ND, TN2], bf16,
                           kind="Internal")

    cshapes = {
        "w0ddT": [128, 128], "dls": [128, 10 * 16], "w1vdd": [20, 16],
        "w2dT": [16, 16 * ND], "w2wT": [16, 16 * NW],
        "wkats": [33, 128 * NW],
        "s8selD": [16 * ND, 16], "s8selW": [16 * NW, 16],
        "vselD": [16 * ND, 128], "vselW": [16 * NW, 128],
        "w3ddTp": [128, 128], "woutddTp": [128, 128], "idd": [128, 128],
    }
    F32_CONSTS = {"b0dd": [128, 1], "cbdd": [16, 1],
                  "bw2D": [16 * ND, 1], "bw2W": [16 * NW, 1],
                  "boutdd": [128, 1], "obc": [16, 128]}
    F32R_CONSTS = {}
    consts_d = {}
    for k_, v in cshapes.items():
        consts_d[k_] = nc.dram_tensor(k_, v, bf16, kind="ExternalInput")
    consts_d["hauginit"] = nc.dram_tensor("hauginit", [33, TN2], bf16,
                                          kind="ExternalInput")
    for k_, v in F32_CONSTS.items():
        consts_d[k_] = nc.dram_tensor(k_, v, f32, kind="ExternalInput")
    for k_, v in F32R_CONSTS.items():
        consts_d[k_] = nc.dram_tensor(k_, v, f32r, kind="ExternalInput")

    # feats arrive k-major per tile ([c, k, n]), so each 512-col chunk is
    # exactly two k-slices and the relu views are fully contiguous.
    XCH = [(i * 512, 512) for i in range(5)]
    XN_ACT = {0, 1, 2, 3}          # chunks relu'd on Act; rest on DVE
    KGROUPS = [(0, 2), (2, 2), (4, 2), (6, 2), (8, 2)]

    with tile.TileContext(nc) as tc:
        with (
            tc.tile_pool(name="singles", bufs=1) as singles,
            tc.tile_pool(name="bigio", bufs=2) as bigio,
            tc.tile_pool(name="work", bufs=2) as work,
            tc.tile_pool(name="small", bufs=2) as small,
            tc.tile_pool(name="xnps", bufs=2, space="PSUM") as xnps_pool,
            tc.tile_pool(name="grpps", bufs=3, space="PSUM") as grpps_pool,
            tc.tile_pool(name="sbank", bufs=1, space="PSUM") as sbank_pool,
        ):
            csb = {}
            for name in cshapes:
                t = singles.tile(cshapes[name], bf16, name=f"c_{name}",
                                 tag=f"c_{name}")
                nc.sync.dma_start(out=t, in_=consts_d[name][:, :])
                csb[name] = t
            for name in F32_CONSTS:
                t = singles.tile(F32_CONSTS[name], f32, name=f"c_{name}",
                                 tag=f"c_{name}")
                nc.sync.dma_start(out=t, in_=consts_d[name][:, :])
                csb[name] = t
            for name in F32R_CONSTS:
                t = singles.tile(F32R_CONSTS[name], f32r, name=f"c_{name}",
                                 tag=f"c_{name}")
                nc.sync.dma_start(out=t, in_=consts_d[name][:, :])
                csb[name] = t

            h_augs = []
            for p in range(2):
                ht = singles.tile([33, TN2], bf16, name=f"haug{p}",
                                  tag=f"haug{p}")
                nc.sync.dma_start(out=ht, in_=consts_d["hauginit"][:, :])
                h_augs.append(ht)

            # persistent PSUM banks
            SBA = sbank_pool.tile([128, 512], f32, name="SBA", tag="SBA")
            SBB = sbank_pool.tile([128, 512], f32, name="SBB", tag="SBB")
            SBC = sbank_pool.tile([128, 512], f32, name="SBC", tag="SBC")
            # HW rules: matmul output base must be 0/32/64 (span <= 32 from
            # base 32); Act/DVE ops keep input and output partition bases
            # EQUAL (shifted bases diverge on real HW vs CoreSim).
            # All PSUM matmul outputs and all Act/DVE in/out partition
            # ranges stay at base 0 (HW is picky about anything else).
            # lgD and lgW time-share SBA[0:96, 0:256]: lgD(j) is consumed by
            # expD before lgW(j) is produced; G and s8 time-share [0:16] of
            # the second half the same way (G -> h relu -> ... -> s8).
            lgD_ps = SBA[0:16 * ND, 0:TN2]
            lgW_ps = SBA[0:16 * NW, 0:TN2]
            G_ps = SBA[0:16, TN2:2 * TN2]
            s8_ps = SBA[0:16, TN2:2 * TN2]
            num_ps = SBB[:, 0:TN2]
            out2_ps = SBB[:, TN2:2 * TN2]
            rsb_ps = [SBC[:, 0:TN2], SBC[:, TN2:2 * TN2]]

            def emit_gates(it):
                """Load + xn + gate pipeline for tile `it` (emitted during
                iteration it-1)."""
                st = {}
                h_aug = h_augs[it % 2]
                feats_t = bigio.tile([128, TN2 * K], bf16, name="feats_t",
                                     tag="feats")
                nc.sync.dma_start(
                    out=feats_t,
                    in_=feats_d[:, it * TN2 * K:(it + 1) * TN2 * K])
                pt1_t = small.tile([16 * ND, TN2], bf16, name="pt1_t",
                                   tag="pt1")
                nc.sync.dma_start(out=pt1_t, in_=pt1_d[it])
                pt2_t = small.tile([16 * NW, TN2], bf16, name="pt2_t",
                                   tag="pt2")
                nc.sync.dma_start(out=pt2_t, in_=pt2_d[it])
                pt20_t = small.tile([20, TN2], bf16, name="pt20_t",
                                    tag="pt20")
                nc.sync.dma_start(out=pt20_t, in_=pt20_d[it])

                # ---- xn = relu(W0 @ feats + b0), k-major bf16 ----
                # bufs=4: gates run two tiles ahead and xn(i) is still read
                # by the head of tile i one iteration after its x3/mult.
                xn_sb = work.tile([128, K, TN2], bf16, name="xn_sb",
                                  tag="xn", bufs=4)
                for ci, (off, sz) in enumerate(XCH):
                    xn_ps = xnps_pool.tile([128, 512], f32, name="xn_ps",
                                           tag="xnps")
                    mm(xn_ps, csb["w0ddT"],
                       feats_t[:, off:off + sz], start=True, stop=True)
                    dst = xn_sb[:, 2 * ci:2 * ci + 2, :]
                    if ci in XN_ACT:
                        nc.scalar.activation(
                            out=dst, in_=xn_ps, func=AF.Relu, bias=csb["b0dd"])
                    else:
                        nc.vector.tensor_scalar(
                            out=dst, in0=xn_ps, scalar1=csb["b0dd"],
                            scalar2=0.0, op0=OP.add, op1=OP.max)
                st["xn_sb"] = xn_sb

                # ---- gate logits G [16, 256] -> h ----
                for j in range(K):
                    mm(G_ps, csb["dls"][:, 16 * j:16 * (j + 1)],
                       xn_sb[:, j, :], start=(j == 0), stop=False)
                mm(G_ps, csb["w1vdd"], pt20_t, start=False, stop=True)
                nc.scalar.activation(
                    out=h_aug[0:16, :], in_=G_ps, func=AF.Relu,
                    bias=csb["cbdd"])

                # ---- compact gate logits + exp (DMA-k's and wkat-k's) ----
                mm(lgD_ps, csb["w2dT"], h_aug[0:16, :], start=True, stop=True)
                eD_sb = small.tile([16 * ND, TN2], bf16, name="eD_sb",
                                   tag="eD")
                nc.scalar.activation(out=eD_sb, in_=lgD_ps, func=AF.Exp,
                                     bias=csb["bw2D"])
                mm(lgW_ps, csb["w2wT"], h_aug[0:16, :], start=True, stop=True)
                eW_sb = small.tile([16 * NW, TN2], bf16, name="eW_sb",
                                   tag="eW")
                nc.scalar.activation(out=eW_sb, in_=lgW_ps, func=AF.Exp,
                                     bias=csb["bw2W"])

                # ---- gate broadcast transport (DMA-k's) ----
                # e-write waits on the Act exp -> issue from Act (free wait);
                # the replicate + re-read have long waits -> issue from the
                # Pool SWDGE queue so no busy sequencer blocks on them.
                nc.scalar.dma_start(out=eflat_d[it], in_=eD_sb)
                rep = eflat_d[it].rearrange("(gh k) n -> gh k n", gh=16)
                rep = rep[:, None, :, :].to_broadcast((16, 8, ND, TN2))
                nc.gpsimd.dma_start(out=e8x_d[it], in_=rep)
                wsD_sb = work.tile([128, ND, TN2], bf16, name="wsD_sb",
                                   tag="wsD", bufs=3)
                nc.gpsimd.dma_start(
                    out=wsD_sb,
                    in_=e8x_d[it].rearrange("gh cc k n -> (gh cc) k n"))
                st["wsD_sb"] = wsD_sb

                # ---- softmax denominator -> rs8 (1/sum via recip) ----
                mm(s8_ps, csb["s8selD"], eD_sb, start=True, stop=False)
                mm(s8_ps, csb["s8selW"], eW_sb, start=False, stop=True)
                rs8_sb = small.tile([16, TN2], f32, name="rs8_sb", tag="rs8")
                scr_sb = small.tile([16, TN2], f32, name="scr_sb", tag="scr")
                nc.vector.reciprocal_approx_accurate(
                    out=rs8_sb, in_=s8_ps, scratch=scr_sb)
                st["rs8_sb"] = rs8_sb

                # ---- eP = e .* ptsn_rep (compact rows) ----
                ePD_sb = small.tile([16 * ND, TN2], bf16, name="ePD_sb",
                                    tag="ePD", bufs=3)
                nc.gpsimd.tensor_tensor(out=ePD_sb, in0=eD_sb,
                                        in1=pt1_t, op=OP.mult)
                ePW_sb = small.tile([16 * NW, TN2], bf16, name="ePW_sb",
                                    tag="ePW", bufs=3)
                nc.gpsimd.tensor_tensor(out=ePW_sb, in0=eW_sb, in1=pt2_t,
                                        op=OP.mult)
                st["ePD_sb"] = ePD_sb
                st["ePW_sb"] = ePW_sb

                # ---- wkat path (PE logit broadcast + Act exp), emitted
                # last so its grp-pool slots reuse x3 slots whose gated
                # mults have had time to drain.
                wsW_sb = work.tile([128, NW, TN2], bf16, name="wsW_sb",
                                   tag="wsW", bufs=3)
                for j0 in range(0, NW, 2):
                    lgw = grpps_pool.tile([128, 2, TN2], f32, name="lgw_ps",
                                          tag="grp")
                    for i in range(2):
                        kk = j0 + i
                        mm(lgw[:, i, :],
                           csb["wkats"][:, 128 * kk:128 * (kk + 1)],
                           h_aug, start=True, stop=True)
                    nc.scalar.activation(
                        out=wsW_sb[:, j0:j0 + 2, :], in_=lgw[:, 0:2, :],
                        func=AF.Exp)
                st["wsW_sb"] = wsW_sb
                return st

            def emit_x3m(it, st):
                """x3 matmuls + gated mults for tile `it`.  Emitted first in
                each iteration so the serial DVE mult chain overlaps the
                gates of tile it+1 on PE/Act."""
                xn_sb = st["xn_sb"]
                y_sb = work.tile([128, TN2, K], bf16, name="y_sb", tag="y")
                yv = y_sb.rearrange("p n k -> p k n")
                for (k0, kg) in KGROUPS:
                    x3_ps = grpps_pool.tile([128, 2, TN2], f32, name="x3_ps",
                                            tag="grp")
                    for i in range(kg):
                        kk = k0 + i
                        mm(x3_ps[:, i, :], csb["w3ddTp"], xn_sb[:, kk, :],
                           start=True, stop=True)
                    if k0 + kg <= ND:
                        ws = st["wsD_sb"][:, k0:k0 + kg, :]
                    else:
                        ws = st["wsW_sb"][:, k0 - ND:k0 - ND + kg, :]
                    nc.vector.tensor_tensor(
                        out=yv[:, k0:k0 + kg, :], in0=ws,
                        in1=x3_ps[:, 0:kg, :], op=OP.mult)
                st["y_sb"] = y_sb

            def emit_redu(it, st):
                """num = sum_k y_k + v (x) sum_k e_k*ptsn_k, PSUM-accumulated."""
                mm(num_ps, csb["vselD"], st["ePD_sb"], start=True, stop=False)
                mm(num_ps, csb["vselW"], st["ePW_sb"], start=False, stop=False)
                y_sb = st["y_sb"]
                for kk in range(K):
                    mm(num_ps, csb["idd"], y_sb[:, :, kk], start=False,
                       stop=(kk == K - 1))

            def emit_head(it, st):
                """Serial head chain of tile `it` (deferred one iteration):
                relu -> normalize -> Wout + residual -> bias -> store."""
                n0 = it * TN
                o1a_sb = small.tile([128, TN2], bf16, name="o1a_sb",
                                    tag="o1a")
                nc.scalar.activation(out=o1a_sb, in_=num_ps, func=AF.Relu,
                                     bias=0.0)
                o1p_sb = small.tile([128, TN2], bf16, name="o1p_sb",
                                    tag="o1p")
                nc.vector.tensor_tensor(out=o1p_sb, in0=o1a_sb,
                                        in1=rsb_ps[it % 2], op=OP.mult)
                mm(out2_ps, csb["woutddTp"], o1p_sb, start=True, stop=False)
                mm(out2_ps, csb["idd"], st["xn_sb"][:, 0, :], start=False,
                   stop=True)
                fin_sb = small.tile([128, TN2], f32, name="fin_sb", tag="fin")
                nc.scalar.activation(out=fin_sb, in_=out2_ps,
                                     func=AF.Identity, bias=csb["boutdd"])
                nc.scalar.dma_start(out=out_d[:, n0:n0 + TN2],
                                    in_=fin_sb[0:64, :])
                nc.scalar.dma_start(out=out_d[:, n0 + TN2:n0 + TN],
                                    in_=fin_sb[64:128, :])

            def emit_obc(it, st):
                mm(rsb_ps[it % 2], csb["obc"], st["rs8_sb"],
                   start=True, stop=True)

            # gates run TWO tiles ahead so the gate-broadcast DMA round trip
            # (~9 us) amortizes over two tile periods.  redu/head of tile
            # it-1 are emitted AFTER x3m(it) so the PE starts each iteration
            # with the x3 groups that feed the serial DVE mult chain.
            states = {0: emit_gates(0), 1: emit_gates(1)}
            emit_obc(0, states[0])
            for it in range(NT):
                emit_x3m(it, states[it])
                if it + 2 < NT:
                    states[it + 2] = emit_gates(it + 2)
                if it == 0:
                    emit_obc(1, states[1])
                else:
                    emit_redu(it - 1, states[it - 1])
                    emit_head(it - 1, states[it - 1])
                    if it + 1 < NT:
                        emit_obc(it + 1, states[it + 1])
                    del states[it - 1]
            emit_redu(NT - 1, states[NT - 1])
            emit_head(NT - 1, states[NT - 1])

    nc.compile()
    return nc


def _fold_weights(inp):
    """Host-side weight folding -> dict of const arrays."""
    W0, b0 = inp["W0"], inp["b0"]
    W1, b1 = inp["W1"], inp["b1"]
    W2, b2 = inp["W2"], inp["b2"]
    W3, b3 = inp["W3"], inp["b3"]
    Wp1, Wp2 = inp["Wp1"], inp["Wp2"]
    Ww1, Ww2, bw2 = inp["Ww1"], inp["Ww2"], inp["bw2"]
    Wout, bout = inp["Wout"], inp["bout"]
    GN = CH // SP

    assert np.abs(b3).max() == 0.0, "kernel folds b3 through the softmax"

    Ww1r = Ww1.reshape(GN, CH, K)
    A = Ww1r.sum(axis=2)
    AW1 = A @ W1
    C2 = np.einsum("omj,mc->ocj", Ww1r, W2)
    Dc = -C2.copy()
    Dc[:, :, 0] += AW1
    cb = A @ (b1 - b2)
    v = Wp2 @ np.maximum(Wp1[:, 0], 0.0)
    w1v = np.einsum("omj,m->oj", Ww1r, v)

    c = {}
    t = np.zeros((128, 128), np.float32)
    t[0:64, 0:64] = W0.T
    t[64:128, 64:128] = W0.T
    c["w0ddT"] = t
    c["b0dd"] = np.concatenate([b0, b0]).reshape(128, 1).astype(np.float32)
    t = np.zeros((128, 10 * 16), np.float32)
    for j in range(K):
        t[0:64, 16 * j:16 * j + 8] = Dc[:, :, j].T
        t[64:128, 16 * j + 8:16 * j + 16] = Dc[:, :, j].T
    c["dls"] = t
    t = np.zeros((20, 16), np.float32)
    for j in range(K):
        t[j, 0:8] = w1v[:, j]
        t[10 + j, 8:16] = w1v[:, j]
    c["w1vdd"] = t
    c["cbdd"] = np.concatenate([cb, cb]).reshape(16, 1).astype(np.float32)

    # compact gate-logit weights: h rows (c_h + 8h) -> pD/pW rows
    w2dT = np.zeros((16, 16 * ND), np.float32)
    bw2D = np.zeros((16 * ND, 1), np.float32)
    w2wT = np.zeros((16, 16 * NW), np.float32)
    bw2W = np.zeros((16 * NW, 1), np.float32)
    for gh in range(16):
        g, h = gh // 2, gh % 2
        for kk in range(ND):
            p = gh * ND + kk
            w2dT[8 * h:8 * h + 8, p] = Ww2[g * K + kk]
            bw2D[p, 0] = bw2[g * K + kk]
        for j in range(NW):
            p = gh * NW + j
            w2wT[8 * h:8 * h + 8, p] = Ww2[g * K + (ND + j)]
            bw2W[p, 0] = bw2[g * K + (ND + j)]
    c["w2dT"] = w2dT
    c["bw2D"] = bw2D
    c["w2wT"] = w2wT
    c["bw2W"] = bw2W

    # wkat blocks for k >= ND, with pi-permuted columns
    t = np.zeros((33, 128 * NW), np.float32)
    for j in range(NW):
        kk = ND + j
        blk = np.zeros((33, 128), np.float32)
        for h in range(2):
            for cc_ in range(CH):
                col = _pi(cc_, h)
                blk[8 * h:8 * h + 8, col] = Ww2[(cc_ % SP) * K + kk]
                blk[32, col] = bw2[(cc_ % SP) * K + kk]
        t[:, 128 * j:128 * (j + 1)] = blk
    c["wkats"] = t

    t = np.zeros((16 * ND, 16), np.float32)
    for gh in range(16):
        g, h = gh // 2, gh % 2
        for kk in range(ND):
            t[gh * ND + kk, g + 8 * h] = 1.0
    c["s8selD"] = t
    t = np.zeros((16 * NW, 16), np.float32)
    for gh in range(16):
        g, h = gh // 2, gh % 2
        for j in range(NW):
            t[gh * NW + j, g + 8 * h] = 1.0
    c["s8selW"] = t

    t = np.zeros((16, 128), np.float32)
    for h in range(2):
        for cc_ in range(CH):
            t[(cc_ % SP) + 8 * h, _pi(cc_, h)] = 1.0
    c["obc"] = t

    t = np.zeros((16 * ND, 128), np.float32)
    for gh in range(16):
        g, h = gh // 2, gh % 2
        for kk in range(ND):
            for cc_ in range(CH):
                if cc_ % SP == g:
                    t[gh * ND + kk, _pi(cc_, h)] = v[cc_]
    c["vselD"] = t
    t = np.zeros((16 * NW, 128), np.float32)
    for gh in range(16):
        g, h = gh // 2, gh % 2
        for j in range(NW):
            for cc_ in range(CH):
                if cc_ % SP == g:
                    t[gh * NW + j, _pi(cc_, h)] = v[cc_]
    c["vselW"] = t

    t = np.zeros((128, 128), np.float32)
    for h in range(2):
        for ci in range(CH):
            for co in range(CH):
                t[ci + 64 * h, _pi(co, h)] = W3[co, ci]
    c["w3ddTp"] = t
    t = np.zeros((128, 128), np.float32)
    for h in range(2):
        for ci in range(CH):
            for co in range(CH):
                t[_pi(ci, h), co + 64 * h] = Wout[co, ci]
    c["woutddTp"] = t
    c["idd"] = np.eye(128, dtype=np.float32)
    t = np.zeros((33, TN2), np.float32)
    t[32, :] = 1.0
    c["hauginit"] = t
    c["boutdd"] = np.concatenate([bout, bout]).reshape(128, 1).astype(
        np.float32)

    F32 = {"b0dd", "cbdd", "bw2D", "bw2W", "boutdd", "obc"}
    for k_ in c:
        c[k_] = np.ascontiguousarray(
            c[k_].astype(np.float32 if k_ in F32 else BF16))
    return c


def make_in_maps(inputs):
    inp = {k: np.ascontiguousarray(np.asarray(v, dtype=np.float32))
           for k, v in inputs.items()}
    consts = _fold_weights(inp)
    cent = inp["cent_pts"]
    spt = inp["sm_pts"]
    ptsn = ((cent.transpose(0, 2, 1)[:, :, :, None] - spt) ** 2).sum(axis=1)
    in_maps = []
    for b in range(B):
        m = dict(consts)
        # k-major per tile-half: feats_d[c + 64h, it*2560 + k*256 + n2]
        ff = inp["sm_feats"][b].reshape(CH, NT, 2, TN2, K)
        ff = ff.transpose(0, 1, 2, 4, 3)            # [c, it, h, k, n2]
        m["feats"] = np.ascontiguousarray(
            np.concatenate([ff[:, :, 0], ff[:, :, 1]], axis=0)
        ).reshape(128, N * K // 2).astype(BF16)
        # ptsn per tile/half: [NT, 2, TN2, K]
        pt = ptsn[b].reshape(NT, 2, TN2, K)
        pt1 = np.zeros((NT, 16 * ND, TN2), np.float32)
        pt2 = np.zeros((NT, 16 * NW, TN2), np.float32)
        pt20 = np.zeros((NT, 20, TN2), np.float32)
        for gh in range(16):
            h = gh % 2
            for kk in range(ND):
                pt1[:, gh * ND + kk, :] = pt[:, h, :, kk]
            for j in range(NW):
                pt2[:, gh * NW + j, :] = pt[:, h, :, ND + j]
        for j in range(K):
            for h in range(2):
                pt20[:, j + 10 * h, :] = pt[:, h, :, j]
        m["pt1"] = np.ascontiguousarray(pt1.astype(BF16))
        m["pt2"] = np.ascontiguousarray(pt2.astype(BF16))
        m["pt20"] = np.ascontiguousarray(pt20.astype(BF16))
        in_maps.append(m)
    return in_maps


def _run(inputs, trace=False):
    from concourse.bass_utils import run_bass_kernel_spmd

    if "nc" not in _CACHE:
        _CACHE["nc"] = _build_bass()
    nc = _CACHE["nc"]
    in_maps = make_in_maps(inputs)

    res = run_bass_kernel_spmd(
        nc, in_maps, core_ids=list(range(B)), trace=trace)
    out = np.stack([r["out"] for r in res.results], axis=0)
    return out, res


def kernel(**inputs) -> np.ndarray:
    out, _ = _run(inputs, trace=False)
    return out
